# revision 88
# baseline (speedup 1.0000x reference)
"""DeepSigNet Trainium2 Bass kernel.

Pure data parallel over 8 NeuronCores (2 batch elements each).

Per stage (3x  signature -> batchnorm -> 2-layer LSTM):
  * depth-3 streamed signature built from row-wise outer products:
      v_j = (sc_j * d + s2'_j) (x) d      (dual-broadcast tensor_tensor)
    and prefix sums realized as PE matmuls in the "flipped" direction
    (v-chunks as stationary lhsT, upper-triangular-ones+weight-col as the
    moving rhs), producing the signature stream S^T with channels on
    partitions and per-channel time-sums fused as a 129th column.
  * per-channel sum-of-squares: ACT Square on the fp16 S + per-chunk
    tensor_scalar accum_out.
  * batch stats all-reduced (one 8-core AllReduce per stage); batchnorm is
    folded into the LSTM input weights (W' = W * invsigma; mean correction
    via an extra matmul column accumulated alongside the gates).
  * the 2-layer LSTM runs as one 129-iteration software-pipelined loop;
    gates live in a single PSUM bank with pair-interleaved (2t+e) columns,
    gate sections at partitions i@0,f@32,o@64,g@96 so that every per-step
    instruction is one uniform [2x2]-pattern op.
"""
import os
import numpy as np
from contextlib import ExitStack

import concourse.bass as bass
import concourse.bacc as bacc
import concourse.tile as tile
from concourse import mybir
from concourse.bass_utils import run_bass_kernel_spmd

F32 = mybir.dt.float32
F16 = mybir.dt.float16
AF = mybir.ActivationFunctionType
OP = mybir.AluOpType

NCORES = int(os.environ.get("KERNEL_NCORES", "8"))
B, L, H = 16, 128, 20
B2 = 2                    # batch elems per core
NB = B2                   # 2
EPS = 1e-5
NSAMP = float(B2 * NCORES * L)

STAGES = []
for C in (29, 20, 20):
    jch = (C * C + 127) // 128          # chunks per j-block (7 / 4)
    STAGES.append(dict(C=C, nj=C, jch=jch, jw=jch * 128,
                       NCH=1 + jch + C * jch))

DEBUG = bool(int(os.environ.get("KERNEL_DEBUG", "0")))
NSTAGES = int(os.environ.get("KERNEL_NSTAGES", "3"))
SKIP_LSTM = bool(int(os.environ.get("KERNEL_SKIP_LSTM", "0")))
SKIP_SIG = bool(int(os.environ.get("KERNEL_SKIP_SIG", "0")))

# ---------------------------------------------------------------------------
# host-side preparation
# ---------------------------------------------------------------------------
_GATE_SRC = [list(range(0, 20)), list(range(20, 40)),
             list(range(60, 80)), list(range(40, 60))]   # i,f,o,g


def _gate_pad_cols(M):
    """(80, K) pytorch-gate-ordered -> (K, 128) padded lhsT."""
    out = np.zeros((M.shape[1], 128), np.float32)
    for s, rows in enumerate(_GATE_SRC):
        out[:, 32 * s:32 * s + 20] = M[rows, :].T
    return out


def _gate_pad_vec(v):
    out = np.zeros(128, np.float32)
    for s, rows in enumerate(_GATE_SRC):
        out[32 * s:32 * s + 20] = v[rows]
    return out


def _chan_map(si):
    """padded channel id -> reference channel id (or -1 for pads)."""
    cfg = STAGES[si]
    C, jch, NCH = cfg['C'], cfg['jch'], cfg['NCH']
    cperm = (np.array([*range(1, 21), 0, *range(21, 29)]) if si == 0
             else np.arange(C))
    m = np.full(NCH * 128, -1, np.int64)
    m[0:C] = cperm[0:C]
    for idx in range(C * C):
        j, k = divmod(idx, C)
        m[128 + idx] = C + cperm[j] * C + cperm[k]
    for j in range(C):
        off = (1 + jch) * 128 + j * cfg['jw']
        for idx in range(C * C):
            k, l = divmod(idx, C)
            m[off + idx] = C + C * C + cperm[j] * C * C + cperm[k] * C + cperm[l]
    return m


def _prep_weights(inputs):
    d = {}
    d['w1T'] = np.ascontiguousarray(np.asarray(inputs['aug_w1']).T)
    d['b1col'] = np.asarray(inputs['aug_b1']).reshape(64, 1).copy()
    d['w2T'] = np.ascontiguousarray(np.asarray(inputs['aug_w2']).T)
    d['b2col'] = np.asarray(inputs['aug_b2']).reshape(8, 1).copy()
    for si, pre in enumerate(('l1', 'l2', 'l3')):
        NCH = STAGES[si]['NCH']
        cm = _chan_map(si)
        Wg = _gate_pad_cols(np.asarray(inputs[pre + '_wih0']))   # (C_tot,128)
        Wpad = np.zeros((NCH * 128, 128), np.float32)
        real = cm >= 0
        Wpad[real, :] = Wg[cm[real], :]
        # double the g-gate section so tanh(g) = 2*sigmoid(2g)-1 works
        # with a single plain sigmoid over all gate sections
        Wpad[:, 96:116] *= 2.0
        # group 4 chunks per DMA transfer: [NCHP/4, 128, 4*128] so each
        # partition row is one contiguous 1KB segment
        NCHP = (NCH + 3) // 4 * 4
        Wq = np.zeros((NCHP, 128, 128), np.float32)
        Wq[:NCH] = Wpad.reshape(NCH, 128, 128)
        d[f'W{si}'] = np.ascontiguousarray(
            Wq.reshape(NCHP // 4, 4, 128, 128).transpose(0, 2, 1, 3)
            .reshape(NCHP // 4, 128, 512)).astype(np.float16)
        whh0 = np.zeros((21, 128), np.float32)
        whh0[0:20] = _gate_pad_cols(np.asarray(inputs[pre + '_whh0']))
        whh0[20] = _gate_pad_vec(np.asarray(inputs[pre + '_bih0']) +
                                 np.asarray(inputs[pre + '_bhh0']))
        whh0[:, 96:116] *= 2.0
        d[f'Whh0T{si}'] = whh0.astype(np.float16)
        w2 = np.zeros((52, 128), np.float32)
        w2[0:20] = _gate_pad_cols(np.asarray(inputs[pre + '_wih1']))
        w2[20] = _gate_pad_vec(np.asarray(inputs[pre + '_bih1']) +
                               np.asarray(inputs[pre + '_bhh1']))
        w2[32:52] = _gate_pad_cols(np.asarray(inputs[pre + '_whh1']))
        w2[:, 96:116] *= 2.0
        d[f'W2T{si}'] = w2.astype(np.float16)
    d['lin_wT'] = np.asarray(inputs['lin_w']).reshape(1, 20).T.copy()
    d['lin_b'] = np.asarray(inputs['lin_b']).reshape(1, 1).copy()
    return d


def _prep_core_inputs(inputs, prep, core):
    inp = np.asarray(inputs['inp'])
    xs = inp[core * B2:(core + 1) * B2]
    aT = np.zeros((21, 258), np.float32)
    for e in range(B2):
        aT[0:20, 1 + e * 129: 129 + e * 129] = xs[e].T
        aT[20, 1 + e * 129: 129 + e * 129] = np.linspace(
            0.0, 1.0, L, dtype=np.float32)
    m = {'aTbase': aT}
    m.update(prep)
    return m


# ---------------------------------------------------------------------------
# kernel builder
# ---------------------------------------------------------------------------

def _ap(t, off_extra, dims, parts=None):
    p = t.ap[0] if parts is None else [t.ap[0][0], parts]
    return bass.AP(tensor=t.tensor, offset=t.offset + off_extra,
                   ap=[p, *dims])


def build_nc():
    nc = bacc.Bacc("TRN2", target_bir_lowering=False, debug=False,
                   num_devices=NCORES)

    aTbase_d = nc.dram_tensor("aTbase", [21, 258], F32, kind="ExternalInput")
    w1T_d = nc.dram_tensor("w1T", [20, 64], F32, kind="ExternalInput")
    b1_d = nc.dram_tensor("b1col", [64, 1], F32, kind="ExternalInput")
    w2T_d = nc.dram_tensor("w2T", [64, 8], F32, kind="ExternalInput")
    b2_d = nc.dram_tensor("b2col", [8, 1], F32, kind="ExternalInput")
    W_d, whh0_d, w2_d = [], [], []
    for si in range(3):
        NCH = STAGES[si]['NCH']
        NCHP = (NCH + 3) // 4 * 4
        W_d.append(nc.dram_tensor(f"W{si}", [NCHP // 4, 128, 512], F16,
                                  kind="ExternalInput"))
        whh0_d.append(nc.dram_tensor(f"Whh0T{si}", [21, 128], F16,
                                     kind="ExternalInput"))
        w2_d.append(nc.dram_tensor(f"W2T{si}", [52, 128], F16,
                                   kind="ExternalInput"))
    linw_d = nc.dram_tensor("lin_wT", [20, 1], F32, kind="ExternalInput")
    linb_d = nc.dram_tensor("lin_b", [1, 1], F32, kind="ExternalInput")
    out_d = nc.dram_tensor("out", [1, NB * L], F32, kind="ExternalOutput")

    dbg = {}
    if DEBUG:
        for nm, shp in [("dbg_tm", [128, 2]), ("dbg_sg", [128, 40]), ("dbg_ct", [52, 524]), ("dbg_gx0", [128, 512]), ("dbg_mh", [128, 512]), ("dbg_d0", [128, 32]), ("dbg_sums", [128, 512]),
                        ("dbg_sumsq", [128, 512]), ("dbg_invs", [128, 512]),
                        ("dbg_gx", [128, 512]), ("dbg_s16", [128, 512]),
                        ("dbg_s2p", [128, 896])]:
            dbg[nm] = nc.dram_tensor(nm, shp, F32, kind="ExternalOutput")
        for nm in ("dbg_ost0", "dbg_ost1", "dbg_ost2"):
            dbg[nm] = nc.dram_tensor(nm, [52, 524], F16,
                                     kind="ExternalOutput")

    ltw = np.zeros((128, 129), np.float32)
    for i in range(128):
        ltw[i, i:128] = 1.0
        ltw[i, 128] = 128 - i
    ltw_d = nc.inline_tensor(ltw.astype(np.float16), name="ltw")
    ltw32_d = nc.inline_tensor(ltw, name="ltw32")
    ident_d = nc.inline_tensor(np.eye(128, dtype=np.float32), name="ident")
    ones_d = nc.inline_tensor(np.ones((1, 524), np.float16), name="onesrow")

    with tile.TileContext(nc) as tc, ExitStack() as ctx:
        glob = ctx.enter_context(tc.tile_pool(name="glob", bufs=1))
        ltw_s = glob.tile([128, 129], F16)
        nc.sync.dma_start(out=ltw_s, in_=ltw_d[:, :])
        ltw32_s = glob.tile([128, 129], F32)
        nc.sync.dma_start(out=ltw32_s, in_=ltw32_d[:, :])
        ident_s = glob.tile([128, 128], F32)
        nc.sync.dma_start(out=ident_s, in_=ident_d[:, :])
        linw_s = glob.tile([20, 1], F32)
        nc.sync.dma_start(out=linw_s, in_=linw_d[:, :])
        linb_s = glob.tile([1, 1], F32)
        nc.sync.dma_start(out=linb_s, in_=linb_d[:, :])
        lstm_w = []
        for si in range(3):
            t0 = glob.tile([21, 128], F16, tag=f"whh0_{si}")
            nc.sync.dma_start(out=t0, in_=whh0_d[si][:, :])
            t1 = glob.tile([52, 128], F16, tag=f"w2_{si}")
            nc.sync.dma_start(out=t1, in_=w2_d[si][:, :])
            lstm_w.append((t0, t1))

        ctile_dbg = []
        lstmp = ctx.enter_context(tc.tile_pool(name="lstmp", bufs=4))
        lstmc = ctx.enter_context(tc.tile_pool(name="lstmc", bufs=1))
        ctile_g = lstmc.tile([52, 524], F32)
        s16_g = lstmc.tile([128, STAGES[0]['NCH'] * NB * 128], F16)
        zerow = lstmc.tile([1, 128], F16)
        nc.vector.memset(zerow[:, :], 0.0)
        sc116 = glob.tile([116, 1], F32)
        nc.vector.memset(sc116[0:96, :], 1.0)
        nc.vector.memset(sc116[96:116, :], 2.0)
        ostacks = [glob.tile([52, 524], F16, tag=f"ost{i}", name=f"ost{i}")
                   for i in range(3)]

        # ================= stage runner =================
        def run_stage(si, d_tiles, ost):
            cfg = STAGES[si]
            C, nj, jch, jw, NCH = (cfg['C'], cfg['nj'], cfg['jch'],
                                   cfg['jw'], cfg['NCH'])
            CC = C * C
            whh0, w2 = lstm_w[si]

            with tc.tile_pool(name=f"gp{si}", bufs=1, space="PSUM") as gp:
                tileG = gp.tile([128, 512], F32)
                tileG2 = gp.tile([128, 512], F32)

                with tc.tile_pool(name=f"stg{si}", bufs=1) as stg:
                    S16 = s16_g
                    bstats = stg.tile([128, NCH * 6], F32)
                    mh = stg.tile([128, NCH], F32)
                    invs = stg.tile([128, NCH], F32)
                    nminv = stg.tile([128, NCH], F32)

                    # ---- per-elem prefix tensors ----
                    prep_cm = tc.tile_pool(name=f"prep{si}", bufs=1)
                    prep = prep_cm.__enter__()
                    sc_t, s2p_t, vu_t = [], [], []
                    with tc.tile_pool(name=f"pp{si}", bufs=2,
                                      space="PSUM") as pp:
                        for e in range(NB):
                            d, _d16 = d_tiles[e]
                            ps1 = pp.tile([128, C], F32, tag="ps1")
                            nc.tensor.matmul(ps1[:, :],
                                             lhsT=ltw32_s[:, 0:128],
                                             rhs=d[:, 0:C], start=True,
                                             stop=True)
                            s1p = prep.tile([128, C], F32, tag=f"s1p{e}")
                            nc.vector.tensor_sub(out=s1p, in0=ps1,
                                                 in1=d[:, 0:C])
                            t05 = prep.tile([128, C], F32, tag=f"t05{e}")
                            nc.vector.scalar_tensor_tensor(
                                out=t05, in0=d[:, 0:C], scalar=0.5, in1=s1p,
                                op0=OP.mult, op1=OP.add)
                            vu = prep.tile([128, jw], F16, tag=f"vu{e}")
                            nc.gpsimd.memset(vu[:, CC:jw], 0.0)
                            nc.vector.tensor_tensor(
                                out=_ap(vu, 0, [[C, C], [1, C]]),
                                in0=_ap(t05, 0, [[1, C], [0, C]]),
                                in1=_ap(d, 0, [[0, C], [1, C]]), op=OP.mult)
                            vu_t.append(vu)
                            # t-part s2 -> s2p
                            nhalf = min(512, jw)
                            ps2a = pp.tile([128, nhalf], F32, tag="ps2a")
                            nc.tensor.matmul(ps2a[:, :], lhsT=ltw_s[:, 0:128],
                                             rhs=vu[:, 0:nhalf], start=True,
                                             stop=True)
                            s2p = prep.tile([128, CC], F32, tag=f"s2p{e}")
                            if jw > 512:
                                ps2b = pp.tile([128, jw - 512], F32,
                                               tag="ps2b")
                                nc.tensor.matmul(ps2b[:, :],
                                                 lhsT=ltw_s[:, 0:128],
                                                 rhs=vu[:, 512:jw],
                                                 start=True, stop=True)
                                nc.vector.tensor_sub(out=s2p[:, 0:512],
                                                     in0=ps2a,
                                                     in1=vu[:, 0:512])
                                nc.vector.tensor_sub(out=s2p[:, 512:CC],
                                                     in0=ps2b[:, 0:CC - 512],
                                                     in1=vu[:, 512:CC])
                            else:
                                nc.vector.tensor_sub(out=s2p,
                                                     in0=ps2a[:, 0:CC],
                                                     in1=vu[:, 0:CC])
                            d6 = prep.tile([128, C], F32, tag=f"d6{e}")
                            nc.vector.tensor_scalar(
                                out=d6, in0=d[:, 0:C], scalar1=1.0 / 6.0,
                                scalar2=None, op0=OP.mult)
                            sc = prep.tile([128, C], F32, tag=f"sc{e}")
                            nc.vector.scalar_tensor_tensor(
                                out=sc, in0=s1p, scalar=0.5, in1=d6,
                                op0=OP.mult, op1=OP.add)
                            sc_t.append(sc)
                            s2p_t.append(s2p)
                            if DEBUG and si == 0 and e == 0:
                                nc.sync.dma_start(out=dbg["dbg_s2p"][:, 0:CC],
                                                  in_=s2p)

                    # ---- split allreduce: half the chunks' stats go out
                    # while the second half is still being generated ----
                    NCH2 = NCH // 2
                    arp_cm = tc.tile_pool(name=f"arp{si}", bufs=1)
                    arp = arp_cm.__enter__()
                    ard_cm = tc.tile_pool(name=f"ard{si}", bufs=1,
                                          space="DRAM")
                    ard = ard_cm.__enter__()

                    def emit_ar(th, g0, g1):
                        n = g1 - g0

                        def bsap(off):
                            return _ap(bstats, g0 * 6 + off, [[6, n]])
                        stats = arp.tile([128, 2 * n], F32, tag=f"st{th}")
                        t1 = arp.tile([128, n], F32, tag=f"t1{th}")
                        u1 = arp.tile([128, n], F32, tag=f"u1{th}")
                        q1 = arp.tile([128, n], F32, tag=f"q1{th}")
                        q2 = arp.tile([128, n], F32, tag=f"q2{th}")
                        nc.vector.tensor_add(out=t1, in0=bsap(1),
                                             in1=bsap(4))
                        nc.vector.tensor_scalar(
                            out=stats[:, 0:n], in0=t1, scalar1=128.0,
                            scalar2=None, op0=OP.mult)
                        nc.vector.tensor_add(out=u1, in0=bsap(2),
                                             in1=bsap(5))
                        nc.gpsimd.tensor_mul(out=q1, in0=bsap(1),
                                             in1=bsap(1))
                        nc.gpsimd.tensor_mul(out=q2, in0=bsap(4),
                                             in1=bsap(4))
                        nc.gpsimd.tensor_add(out=q1, in0=q1, in1=q2)
                        nc.vector.scalar_tensor_tensor(
                            out=stats[:, n:2 * n], in0=q1, scalar=128.0,
                            in1=u1, op0=OP.mult, op1=OP.add)
                        bIn = ard.tile([128, 2 * n], F32, tag=f"bi{th}")
                        bOut = ard.tile([128, 2 * n], F32, tag=f"bo{th}")
                        nc.sync.dma_start(out=bIn[:, :], in_=stats)
                        nc.gpsimd.collective_compute(
                            "AllReduce", OP.add,
                            replica_groups=[list(range(NCORES))],
                            ins=[bIn[:, :].opt()], outs=[bOut[:, :].opt()])
                        statsr = arp.tile([128, 2 * n], F32, tag=f"sr{th}")
                        nc.sync.dma_start(out=statsr, in_=bOut[:, :])
                        mean = arp.tile([128, n], F32, tag=f"mn{th}")
                        nc.vector.tensor_scalar(
                            out=mean, in0=statsr[:, 0:n],
                            scalar1=1.0 / NSAMP, scalar2=None, op0=OP.mult)
                        var = arp.tile([128, n], F32, tag=f"vr{th}")
                        nc.vector.tensor_scalar(
                            out=var, in0=statsr[:, n:2 * n],
                            scalar1=1.0 / NSAMP, scalar2=None, op0=OP.mult)
                        m2 = arp.tile([128, n], F32, tag=f"m2{th}")
                        nc.vector.tensor_mul(out=m2, in0=mean, in1=mean)
                        nc.vector.tensor_sub(out=var, in0=var, in1=m2)
                        ve = arp.tile([128, n], F32, tag=f"ve{th}")
                        nc.vector.tensor_scalar(out=ve, in0=var, scalar1=EPS,
                                                scalar2=None, op0=OP.add)
                        rt = arp.tile([128, n], F32, tag=f"rt{th}")
                        nc.scalar.activation(out=rt, in_=ve, func=AF.Sqrt)
                        x0 = arp.tile([128, n], F32, tag=f"x0{th}")
                        nc.vector.reciprocal(out=x0, in_=rt)
                        x2 = arp.tile([128, n], F32, tag=f"x2{th}")
                        nc.vector.tensor_mul(out=x2, in0=x0, in1=x0)
                        nc.vector.tensor_mul(out=x2, in0=ve, in1=x2)
                        nc.vector.tensor_scalar(out=x2, in0=x2, scalar1=-0.5,
                                                scalar2=1.5, op0=OP.mult,
                                                op1=OP.add)
                        nc.vector.tensor_mul(out=invs[:, g0:g1], in0=x0,
                                             in1=x2)
                        nc.vector.tensor_copy(out=mh[:, g0:g1], in_=mean)
                        nc.vector.scalar_tensor_tensor(
                            out=nminv[:, g0:g1], in0=mean, scalar=-1.0,
                            in1=invs[:, g0:g1], op0=OP.mult, op1=OP.mult)

                    # ---- unified slot stream: cumsum + copy + stats ----
                    # slot s = g*NB + e ; lhsT source per (g, e)
                    with tc.tile_pool(name=f"sig{si}", bufs=4) as sg, \
                         tc.tile_pool(name=f"psig{si}", bufs=3,
                                      space="PSUM") as pg:
                        vjs = {}
                        GRP = 3
                        nslots = NCH * NB if not SKIP_SIG else 0
                        psJ = None
                        next_bn = 0

                        def build_vj(j, e2):
                            dte = d_tiles[e2][0]
                            Mj = sg.tile([128, C], F32, tag=f"Mj{e2}")
                            nc.vector.scalar_tensor_tensor(
                                out=Mj, in0=dte[:, 0:C],
                                scalar=sc_t[e2][:, j:j + 1],
                                in1=s2p_t[e2][:, C * j:C * j + C],
                                op0=OP.mult, op1=OP.add)
                            vj = sg.tile([128, jw], F16, tag=f"vj{e2}")
                            nc.gpsimd.memset(vj[:, CC:jw], 0.0)
                            nc.gpsimd.tensor_tensor(
                                out=_ap(vj, 0, [[C, C], [1, C]]),
                                in0=_ap(Mj, 0, [[1, C], [0, C]]),
                                in1=_ap(dte, 0, [[0, C], [1, C]]),
                                op=OP.mult)
                            return vj
                        for s in range(nslots):
                            g, e = divmod(s, NB)
                            if g == 0:
                                lhsT = d_tiles[e][1][:, 0:128]
                            elif g <= jch:
                                lhsT = vu_t[e][:, 128 * (g - 1):128 * g]
                            else:
                                j, gg = divmod(g - 1 - jch, jch)
                                if j == 0 and gg == 0 and e == 0:
                                    for e2 in range(NB):
                                        vjs[(0, e2)] = build_vj(0, e2)
                                if gg == 0 and e == 0 and j + 1 < nj:
                                    # build next block's v_j on gpsimd while
                                    # this block's slots run on the PE
                                    for e2 in range(NB):
                                        vjs[(j + 1, e2)] = build_vj(j + 1, e2)
                                lhsT = vjs[(j, e)][:, 128 * gg:128 * (gg + 1)]
                            gi = s % GRP
                            if gi == 0:
                                psJ = pg.tile([128, GRP, 128], F32, tag="psJ")
                            nc.tensor.matmul(psJ[:, gi, :], lhsT=lhsT,
                                             rhs=ltw_s[:, 0:128], start=True,
                                             stop=True)
                            if gi == GRP - 1 or s == nslots - 1:
                                n = gi + 1
                                s0 = s - gi
                                nc.scalar.activation(
                                    out=_ap(S16, s0 * 128, [[1, n * 128]]),
                                    in_=_ap(psJ, 0, [[128, n], [1, 128]]),
                                    func=AF.Copy)
                                # per-chunk batchnorm stats from fp16 S
                                while next_bn < (s + 1) // 2:
                                    g_ = next_bn
                                    nc.vector.bn_stats(
                                        out=_ap(bstats, g_ * 6, [[1, 6]]),
                                        in_=_ap(S16, g_ * NB * 128,
                                                [[1, NB * 128]]))
                                    next_bn += 1
                                    if next_bn == NCH2:
                                        emit_ar("a", 0, NCH2)

                    # finish stats for the second half of the chunks
                    emit_ar("b", NCH2, NCH)
                    ard_cm.__exit__(None, None, None)
                    arp_cm.__exit__(None, None, None)
                    prep_cm.__exit__(None, None, None)

                    # ---- gates matmul (fp16); W already resident in SBUF;
                    # per-chunk center+normalize interleaved. Stage 0 only
                    # computes steps 0:64 here (cols 0:128); the remaining
                    # columns are accumulated inside the LSTM loop in the
                    # tensor engine's idle windows. start=True clears the
                    # whole bank's has_written, so the bank is primed once
                    # and every data matmul uses start=False. ----
                    split0 = (si == 0) and bool(int(
                        os.environ.get("KERNEL_SPLIT0", "0")))
                    ncolsA = 128 if split0 else 256
                    nc.tensor.matmul(
                        tileG[:, 0:256], lhsT=zerow[0:1, :],
                        rhs=_ap(S16, 0, [[1, 256]], parts=1),
                        start=True, stop=True, skip_group_check=True)
                    GB = 4
                    with tc.tile_pool(name=f"wr{si}", bufs=1) as wrp:
                        wr4 = None
                        for g in range(NCH):
                            if g % GB == 0:
                                wr4 = wrp.tile([128, GB * 128], F16,
                                               tag="wring", bufs=5)
                                nc.sync.dma_start(
                                    out=wr4, in_=W_d[si][g // GB, :, :])
                            sl = _ap(S16, g * NB * 128, [[1, NB * 128]])
                            if g % 2 == 0:
                                nc.vector.tensor_scalar(
                                    out=sl, in0=sl,
                                    scalar1=mh[:, g:g + 1],
                                    scalar2=invs[:, g:g + 1],
                                    op0=OP.subtract, op1=OP.mult)
                            else:
                                nc.scalar.activation(
                                    out=sl, in_=sl, func=AF.Identity,
                                    scale=invs[:, g:g + 1],
                                    bias=nminv[:, g:g + 1])
                            nc.tensor.matmul(
                                tileG[:, 0:ncolsA],
                                lhsT=wr4[:, (g % GB) * 128:
                                         (g % GB) * 128 + 128],
                                rhs=_ap(S16, g * NB * 128,
                                        [[1, ncolsA // NB], [128, NB]]),
                                start=False, stop=True,
                                skip_group_check=True)
                    passB = list(range(NCH)) if split0 else []
                    if DEBUG and si == 0:
                        with tc.tile_pool(name="dbggx", bufs=1) as dgp:
                            gxs = dgp.tile([128, 256], F32)
                            nc.vector.tensor_copy(out=gxs,
                                                  in_=tileG[:, 0:256])
                            nc.sync.dma_start(out=dbg["dbg_gx"][:, 0:256],
                                              in_=gxs)
                            s16s = dgp.tile([128, 256], F32)
                            nc.vector.tensor_copy(
                                out=s16s, in_=_ap(S16, 0, [[1, 256]]))
                            nc.sync.dma_start(out=dbg["dbg_s16"][:, 0:256],
                                              in_=s16s)

                # stg pool closed: S16/W16 freed. LSTM uses tileG only.
                # Two layers run as separate per-layer chains (L2 lags by
                # one step) so the engines pipeline; the step-k L1 gates
                # accumulate start=False onto the precomputed input gates
                # still sitting in PSUM (has_written persists).
                if True:
                    lp = lstmp
                    ctile = ctile_g
                    nc.vector.memset(ctile[32:52, :], 0.0)
                    nc.vector.memset(ost[0:52, :], 0.0)
                    nc.sync.dma_start(out=ost[20:21, :], in_=ones_d[:, :])

                    def cell_pre(tag, gt, base):
                        # one sigmoid over all gate sections; the g section's
                        # weights are pre-doubled so tanh(g) = 2*sig(2g)-1
                        sg = lp.tile([116, 2], F32, tag=f"sg{tag}")
                        gk = lp.tile([20, 2], F32, tag=f"gk{tag}")
                        nc.scalar.activation(
                            out=sg[0:116, :], in_=gt[0:116, base:base + 2],
                            func=AF.Sigmoid)
                        nc.vector.tensor_scalar(
                            out=gk[0:20, :], in0=sg[96:116, :],
                            scalar1=2.0, scalar2=-1.0,
                            op0=OP.mult, op1=OP.add)
                        return sg, gk

                    def cell_mid(tag, sg, gk, cb):
                        t2 = lp.tile([52, 2], F32, tag=f"t2{tag}")
                        tmp = lp.tile([52, 2], F32, tag=f"tm{tag}")
                        nc.gpsimd.tensor_mul(out=t2[32:52, :],
                                             in0=sg[32:52, :],
                                             in1=ctile[32:52, cb:cb + 2])
                        nc.vector.tensor_mul(out=tmp[32:52, :],
                                             in0=sg[0:20, :], in1=gk[0:20, :])
                        nc.vector.tensor_add(out=ctile[32:52, cb + 2:cb + 4],
                                             in0=t2[32:52, :],
                                             in1=tmp[32:52, :])

                    def cell_post(tag, sg, cb, hr0, hr1, hcol):
                        tau = lp.tile([84, 2], F32, tag=f"ta{tag}")
                        nc.scalar.activation(
                            out=tau[64:84, :],
                            in_=ctile[32:52, cb + 2:cb + 4], func=AF.Tanh)
                        nc.gpsimd.tensor_mul(out=ost[hr0:hr1, hcol:hcol + 2],
                                             in0=sg[64:84, :],
                                             in1=tau[64:84, :])

                    pb_mm = 0
                    pb_dma = 0
                    wrB_q = []
                    post_b_prev = None
                    for k in range(0 if not SKIP_LSTM else L + 1, L + 1):
                        doL1 = k <= L - 1
                        doL2 = k >= 1
                        # flush the one-iter-delayed L2 tail first: mm2
                        # below reads the h2 column it writes
                        if post_b_prev is not None:
                            cell_post("b", *post_b_prev)
                            post_b_prev = None
                        if doL1:
                            nc.tensor.matmul(
                                tileG[:, 2 * k:2 * k + 2], lhsT=whh0,
                                rhs=ost[0:21, 2 * k:2 * k + 2],
                                start=False, stop=True, skip_group_check=True)
                        if doL2:
                            nc.tensor.matmul(
                                tileG2[:, 2 * k:2 * k + 2], lhsT=w2,
                                rhs=ost[0:52, 2 * k:2 * k + 2],
                                start=True, stop=True, skip_group_check=True)
                        if passB and k >= 1:
                            # stage-0 gates for steps 64:128 accumulate in
                            # the tensor engine's idle window of each iter;
                            # weights re-streamed with DMA group lookahead
                            ngroups = (len(passB) + 3) // 4
                            while pb_dma < ngroups and len(wrB_q) < 3:
                                wt = lp.tile([128, 512], F16, tag="wrB",
                                             bufs=4)
                                nc.sync.dma_start(
                                    out=wt, in_=W_d[si][pb_dma, :, :])
                                wrB_q.append(wt)
                                pb_dma += 1
                            if wrB_q and k >= 2:
                                wt = wrB_q.pop(0)
                                for j in range(4):
                                    g_ = pb_mm + j
                                    if g_ >= len(passB):
                                        break
                                    nc.tensor.matmul(
                                        tileG[:, 128:256],
                                        lhsT=wt[:, j * 128:j * 128 + 128],
                                        rhs=_ap(S16, g_ * NB * 128 + 64,
                                                [[1, 64], [128, NB]]),
                                        start=False, stop=True,
                                        skip_group_check=True)
                                pb_mm += 4
                                if pb_mm >= len(passB):
                                    passB = []
                        sga = gka = sgb = gkb = None
                        if doL1:
                            sga, gka = cell_pre("a", tileG, 2 * k)
                            cell_mid("a", sga, gka, 2 * k)
                        if doL2:
                            sgb, gkb = cell_pre("b", tileG2, 2 * k)
                            cell_mid("b", sgb, gkb, 260 + 2 * k)
                        if doL1:
                            cell_post("a", sga, 2 * k, 0, 20, 2 * k + 2)
                        # L2's tail is pipelined one iteration behind (its
                        # write is flushed at the top of the next iter);
                        # this keeps tau-c2 from blocking the next sigma
                        post_b_prev = ((sgb, 260 + 2 * k, 32, 52, 2 * k + 2)
                                       if doL2 else None)
                    if post_b_prev is not None:
                        cell_post("b", *post_b_prev)
                        if DEBUG and si == 0 and k in (0, 1, 2):
                            nc.sync.dma_start(
                                out=dbg["dbg_sg"][0:84, 10 * k:10 * k + 2],
                                in_=sga[0:84, :])
                            nc.sync.dma_start(
                                out=dbg["dbg_sg"][96:116,
                                                  10 * k:10 * k + 2],
                                in_=gka[0:20, :])
                    if DEBUG and si == 0:
                        nc.sync.dma_start(out=dbg["dbg_ct"][:, :],
                                          in_=ctile[:, :])
                        with tc.tile_pool(name="dbgg2", bufs=1) as dg2:
                            g2 = dg2.tile([128, 512], F32)
                            nc.vector.tensor_copy(out=g2, in_=tileG[:, :])
                            nc.sync.dma_start(out=dbg["dbg_gx0"][:, :],
                                              in_=g2)

        # ================= stage 0 prep =================
        d0_tiles = []
        with tc.tile_pool(name="p0", bufs=1) as p0, \
             tc.tile_pool(name="pp0", bufs=2, space="PSUM") as pp0:
            aT = p0.tile([21, 258], F32)
            nc.sync.dma_start(out=aT, in_=aTbase_d[:, :])
            w1T = p0.tile([20, 64], F32)
            nc.sync.dma_start(out=w1T, in_=w1T_d[:, :])
            b1 = p0.tile([64, 1], F32)
            nc.sync.dma_start(out=b1, in_=b1_d[:, :])
            w2T = p0.tile([64, 8], F32)
            nc.sync.dma_start(out=w2T, in_=w2T_d[:, :])
            b2 = p0.tile([8, 1], F32)
            nc.sync.dma_start(out=b2, in_=b2_d[:, :])
            ph = pp0.tile([64, 256], F32)
            nc.tensor.matmul(ph[:, :], lhsT=w1T,
                             rhs=_ap(aT[0:20, :], 1, [[129, 2], [1, 128]]),
                             start=True, stop=True)
            r1 = p0.tile([64, 256], F32)
            nc.scalar.activation(out=r1, in_=ph, func=AF.Relu,
                                 bias=b1[:, 0:1])
            ph2 = pp0.tile([8, 256], F32)
            nc.tensor.matmul(ph2[:, :], lhsT=w2T, rhs=r1, start=True,
                             stop=True)
            hidT = p0.tile([8, 258], F32)
            nc.vector.memset(hidT[:, :], 0.0)
            nc.vector.tensor_scalar(
                out=_ap(hidT, 1, [[129, 2], [1, 128]]), in0=ph2,
                scalar1=b2[:, 0:1], scalar2=None, op0=OP.add)
            dT = p0.tile([21, 256], F32)
            nc.vector.tensor_sub(
                out=_ap(dT, 0, [[128, 2], [1, 128]]),
                in0=_ap(aT, 1, [[129, 2], [1, 128]]),
                in1=_ap(aT, 0, [[129, 2], [1, 128]]))
            dhT = p0.tile([8, 256], F32)
            nc.vector.tensor_sub(
                out=_ap(dhT, 0, [[128, 2], [1, 128]]),
                in0=_ap(hidT, 1, [[129, 2], [1, 128]]),
                in1=_ap(hidT, 0, [[129, 2], [1, 128]]))
            for e in range(NB):
                dt_ = glob.tile([128, 128], F32, tag=f"d0_{e}")
                nc.vector.memset(dt_[:, :], 0.0)
                pt = pp0.tile([128, 21], F32, tag="pt")
                nc.tensor.transpose(pt[:, :],
                                    in_=dT[0:21, 128 * e:128 * (e + 1)],
                                    identity=ident_s[0:21, 0:21])
                nc.vector.tensor_copy(out=dt_[:, 0:21], in_=pt[:, :])
                pt2 = pp0.tile([128, 8], F32, tag="pt2")
                nc.tensor.transpose(pt2[:, :],
                                    in_=dhT[0:8, 128 * e:128 * (e + 1)],
                                    identity=ident_s[0:8, 0:8])
                nc.vector.tensor_copy(out=dt_[:, 21:29], in_=pt2[:, :])
                dt16 = glob.tile([128, 128], F16, tag=f"d0h_{e}")
                nc.vector.tensor_copy(out=dt16, in_=dt_)
                d0_tiles.append((dt_, dt16))
                if DEBUG and e == 0:
                    nc.sync.dma_start(out=dbg["dbg_d0"][:, 0:29],
                                      in_=dt_[:, 0:29])

        def prep_next(ost_prev, si):
            dts = []
            with tc.tile_pool(name=f"pn{si}", bufs=1) as pn, \
                 tc.tile_pool(name=f"ppn{si}", bufs=2, space="PSUM") as ppn:
                dT = pn.tile([20, 256], F32)
                nc.vector.tensor_sub(
                    out=_ap(dT, 0, [[128, 2], [1, 128]]),
                    in0=_ap(ost_prev[32:52, :], 4, [[1, 2], [2, 128]]),
                    in1=_ap(ost_prev[32:52, :], 2, [[1, 2], [2, 128]]))
                for e in range(NB):
                    dt_ = glob.tile([128, 128], F32, tag=f"d{si}_{e}")
                    nc.vector.memset(dt_[:, :], 0.0)
                    pt = ppn.tile([128, 20], F32, tag="pt")
                    nc.tensor.transpose(pt[:, :],
                                        in_=dT[0:20, 128 * e:128 * (e + 1)],
                                        identity=ident_s[0:20, 0:20])
                    nc.vector.tensor_copy(out=dt_[:, 0:20], in_=pt[:, :])
                    dt16 = glob.tile([128, 128], F16, tag=f"d{si}h_{e}")
                    nc.vector.tensor_copy(out=dt16, in_=dt_)
                    dts.append((dt_, dt16))
            return dts

        run_stage(0, d0_tiles, ostacks[0])
        if DEBUG:
            nc.sync.dma_start(out=dbg["dbg_ost0"][:, :], in_=ostacks[0])
        if NSTAGES >= 2:
            d1_tiles = prep_next(ostacks[0], 1)
            run_stage(1, d1_tiles, ostacks[1])
        if DEBUG:
            nc.sync.dma_start(out=dbg["dbg_ost1"][:, :], in_=ostacks[1])
        if NSTAGES >= 3:
            d2_tiles = prep_next(ostacks[1], 2)
            run_stage(2, d2_tiles, ostacks[2])
        if DEBUG:
            nc.sync.dma_start(out=dbg["dbg_ost2"][:, :], in_=ostacks[2])

        # ---- final leaky relu + linear ----
        with tc.tile_pool(name="fin", bufs=1) as fin, \
             tc.tile_pool(name="pfin", bufs=1, space="PSUM") as pfin:
            o3 = fin.tile([20, 256], F32)
            nc.vector.tensor_copy(
                out=_ap(o3, 0, [[128, 2], [1, 128]]),
                in_=_ap(ostacks[NSTAGES - 1][32:52, :], 4,
                        [[1, 2], [2, 128]]))
            ol = fin.tile([20, 256], F32)
            nc.vector.tensor_scalar(out=ol, in0=o3, scalar1=0.01,
                                    scalar2=None, op0=OP.mult)
            nc.vector.tensor_max(out=ol, in0=ol, in1=o3)
            pf = pfin.tile([1, 256], F32)
            nc.tensor.matmul(pf[:, :], lhsT=linw_s, rhs=ol, start=True,
                             stop=True)
            ob = fin.tile([1, 256], F32)
            nc.vector.tensor_scalar(out=ob, in0=pf,
                                    scalar1=linb_s[0:1, 0:1],
                                    scalar2=None, op0=OP.add)
            nc.sync.dma_start(out=out_d[:, :], in_=ob)

    nc.compile()
    return nc


_NC_CACHE = {}


def kernel(**inputs):
    if 'nc' not in _NC_CACHE:
        _NC_CACHE['nc'] = build_nc()
    nc = _NC_CACHE['nc']
    prep = _prep_weights(inputs)
    in_maps = [_prep_core_inputs(inputs, prep, c) for c in range(NCORES)]
    kw = {}
    if os.environ.get("KERNEL_TRACE"):
        kw = dict(trace=True,
                  tmpdir=os.environ.get("KERNEL_TRACE_DIR") or None)
    res = run_bass_kernel_spmd(nc, in_maps, list(range(NCORES)), **kw)
    _NC_CACHE['res'] = res
    outs = [np.asarray(res.results[c]['out']).reshape(B2, L, 1)
            for c in range(NCORES)]
    return np.concatenate(outs, axis=0).astype(np.float32)

